# revision 1
# baseline (speedup 1.0000x reference)
"""Trainium2 Bass kernel for nn_AttentionScore (sparse local attention scores).

Reference computation (B=4, C=64, N=16384, S=16):
    tmp   = xyz[:, :, :, None] - neighbor_xyz            # [B,3,N,S]
    pos   = concat([tmp, ||tmp||], axis=1)               # [B,4,N,S]
    k     = Wk @ (neighbor_points + Wpos @ pos + bpos)   # [B,C,N,S]
    attn  = softmax_s((points*scale) . k)                # [B,N,S]

Softmax over s is shift-invariant, so every term constant in s drops out:
    attn[m,s] ~ sum_c qW[c,m]*np[c,m,s] + sum_j qp[j,m]*tmp[j,m,s] + qp3[m]*||tmp||
with qW = (scale*Wk)^T @ points, qp = Wpos^T @ qW (bpos and the xyz.qp dot cancel).

Sharding: N split contiguously across 8 cores (no communication needed).
m = b*2048 + n_local in [0, 8192) per core, split in halves h = m // 4096.

Main-term dataflow per core:
  - np staged as [128 part = (h,c), (mm,s)] tiles; DVE multiplies by qW
    broadcast over s; TensorE reduces the 64 c-partitions per half with a
    block-ones [128,2] matmul (4x col-tiled into PSUM partitions 32j+h);
    ScalarE copies PSUM->SBUF; a partition-scatter SBUF->SBUF DMA lands
    results in the softmax layout [p = m//64, (m%64)*16+s].
"""

import os
import sys

sys.path.insert(0, "/opt/trn_rl_repo")

import numpy as np

import concourse.bass as bass
import concourse.bacc as bacc
import concourse.tile as tile
from concourse import mybir
from concourse.bass_utils import run_bass_kernel_spmd

F32 = mybir.dt.float32
F32R = mybir.dt.float32r
BF16 = mybir.dt.bfloat16
AF = mybir.ActivationFunctionType
AX = mybir.AxisListType
OP = mybir.AluOpType

B, C, N, S = 4, 64, 16384, 16
NCORES = 8
NL = N // NCORES            # 2048 points per core
M = B * NL                  # 8192 (b, n) rows per core
MH = M // 2                 # 4096 rows per half
MB = 256                    # mm per supertile (per half)
NT = MH // MB               # 16 supertiles
SCALE = float(C) ** -0.5

# float32r streams the moving operand at 1 col/cycle (vs 4 for fp32) and is
# more precise than the fp32 emulation path. Used for the big channel
# reduction only; small matmuls (qW, qp) stay fp32.
USE_F32R_REDUCE = True


def _body(tc):
    nc = tc.nc
    dma = nc.sync.dma_start

    NP = nc.dram_tensor("NP", [128, MH * S], F32, kind="ExternalInput").ap()
    NX = nc.dram_tensor("NX", [128, (M // 128) * 3 * S], F32, kind="ExternalInput").ap()
    XYZ = nc.dram_tensor("XYZ", [128, (M // 128) * 3], F32, kind="ExternalInput").ap()
    P = nc.dram_tensor("P", [C, M], F32, kind="ExternalInput").ap()
    WK = nc.dram_tensor("WK", [C, C], F32, kind="ExternalInput").ap()
    WKT = nc.dram_tensor("WKT", [C, C], F32, kind="ExternalInput").ap()
    WP = nc.dram_tensor("WP", [C, 4], F32, kind="ExternalInput").ap()
    OUT = nc.dram_tensor("OUT", [128, (M // 128) * S], F32, kind="ExternalOutput").ap()

    RDT = F32R if USE_F32R_REDUCE else F32

    with (
        tc.tile_pool(name="const", bufs=1) as cp,
        tc.tile_pool(name="w3072", bufs=2) as p3072,
        tc.tile_pool(name="w1024", bufs=3) as p1024,
        tc.tile_pool(name="small", bufs=1) as sp,
        # main-loop pools open up-front so NP prefetch DMAs have their SBUF
        # space from the start and fully overlap phase 1/2
        tc.tile_pool(name="npt", bufs=3) as npp,
        tc.tile_pool(name="prod", bufs=2) as prp,
        tc.tile_pool(name="sc", bufs=2) as scp,
        tc.tile_pool(name="psm", bufs=2, space="PSUM") as psm,
    ):
        # ---- constant loads ----
        wk = cp.tile([C, C], F32)
        dma(wk[:], WK)
        wkt = cp.tile([C, C], F32)
        dma(wkt[:], WKT)
        wp = cp.tile([C, 4], F32)
        dma(wp[:], WP)
        nxt = cp.tile([128, 64 * 3 * S], F32)
        dma(nxt[:], NX)
        xyzt = cp.tile([128, 64 * 3], F32)
        dma(xyzt[:], XYZ)

        wks = sp.tile([C, C], F32)
        nc.vector.tensor_scalar_mul(wks[:], wk[:], SCALE)
        wkts = sp.tile([C, C], F32)
        nc.vector.tensor_scalar_mul(wkts[:], wkt[:], SCALE)

        # Per-chunk half-selectors: lhsT for chunk k is hs[:, k*16:(k+1)*16],
        # whose column h*8+k is 1 on the h-half partitions. The 8 chunk
        # matmuls accumulate into one [16, 512] PSUM tile with chunk k's
        # half-h sums landing on row h*8+k (other rows accumulate zeros).
        # Built in f32 and copied with an f32r-rounding DVE op so the
        # fp32r matmul sees a properly "rounded" producer.
        hs0 = sp.tile([128, 16 * 8], F32)
        nc.vector.memset(hs0[:], 0.0)
        for k in range(8):
            nc.vector.memset(hs0[0:64, k * 16 + k:k * 16 + k + 1], 1.0)
            nc.vector.memset(hs0[64:128, k * 16 + 8 + k:k * 16 + 8 + k + 1], 1.0)
        hs = sp.tile([128, 16 * 8], RDT)
        nc.vector.tensor_copy(hs[:], hs0[:])

        qw = cp.tile([128, MH], F32)      # row h*64+c holds qW[c, h*MH + mm]
        qpt = cp.tile([128, 4 * 64], F32)  # row p, col j*64+mi: qp[j, p*64+mi]
        attn1 = cp.tile([128, 64 * S], F32)
        attn2 = cp.tile([128, 64 * S], F32)

        # ---- phase 1: qW / qp via bf16 Karatsuba on the PE ----
        # X @ Y ~= Xh@Yh + Xh@Yl + Xl@Yh with h/l the bf16 split; ~2^-18
        # relative error at 1 cycle/col (vs 4 for the fp32 emulation).
        # Chunked q keeps SBUF small so NP prefetch overlaps phase 1; (h0,h1)
        # chunk pairs emit in cc order so early supertiles unblock first.
        CH = 512
        NC1 = M // CH
        with (
            tc.tile_pool(name="qchunk", bufs=2) as qcp,
            tc.tile_pool(name="qps_p", bufs=2) as qpsp,
            tc.tile_pool(name="psq", bufs=2, space="PSUM") as psq,
            tc.tile_pool(name="psp", bufs=2, space="PSUM") as psp,
            tc.tile_pool(name="psw", bufs=1, space="PSUM") as psw,
        ):
            # Wkp[c, j] = sum_c' (scale*Wk)[c, c'] Wpos[c', j]  (fp32, tiny)
            pwkp = psw.tile([C, 4], F32)
            nc.tensor.matmul(pwkp[:], lhsT=wkts[:], rhs=wp[:], start=True, stop=True)
            wkp = sp.tile([C, 4], F32)
            nc.scalar.copy(wkp[:], pwkp[:])

            # Zero-padded qW weights: block h is [64, 128] with cols
            # h*64..h*64+64 = scale*Wk, so out rows h*64.. hold qW and every
            # PSUM partition is written. Split into bf16 hi/lo.
            wkh0 = sp.tile([C, 2 * 128], F32)
            nc.vector.memset(wkh0[:], 0.0)
            nc.vector.tensor_copy(wkh0[:, 0:64], wks[:])
            nc.vector.tensor_copy(wkh0[:, 192:256], wks[:])
            whh = sp.tile([C, 2 * 128], BF16)
            nc.vector.tensor_copy(whh[:], wkh0[:])
            whl0 = sp.tile([C, 2 * 128], F32)
            nc.vector.tensor_sub(whl0[:], wkh0[:], whh[:])
            whl = sp.tile([C, 2 * 128], BF16)
            nc.vector.tensor_copy(whl[:], whl0[:])

            wkph = sp.tile([C, 4], BF16)
            nc.vector.tensor_copy(wkph[:], wkp[:])
            wkpl0 = sp.tile([C, 4], F32)
            nc.vector.tensor_sub(wkpl0[:], wkp[:], wkph[:])
            wkpl = sp.tile([C, 4], BF16)
            nc.vector.tensor_copy(wkpl[:], wkpl0[:])

            # (h0, h1) chunk pairs in cc order so qw columns needed by the
            # first supertiles are produced first.
            qps_tiles = {}
            qps_fill = {}
            for cc in range(NC1 // 2):
              for h in range(2):
                t = h * (NC1 // 2) + cc
                rows = slice(h * 64, h * 64 + 64)
                wsl = slice(h * 128, (h + 1) * 128)

                qf = qcp.tile([C, CH], F32, tag="qf")
                # first pair rides the (empty) Sync queue ahead of NP tile 0
                # so supertile 0's qW dependency clears early; later chunks
                # go through SWDGE to keep Sync free for NP prefetch.
                if cc == 0:
                    dma(qf[:], P[:, t * CH:(t + 1) * CH])
                else:
                    nc.gpsimd.dma_start(qf[:], P[:, t * CH:(t + 1) * CH])
                qhh = qcp.tile([C, CH], BF16, tag="qhh")
                nc.scalar.copy(qhh[:], qf[:])
                qll = qcp.tile([C, CH], BF16, tag="qll")
                nc.vector.tensor_sub(qll[:], qf[:], qhh[:])

                cc2 = (t % (NC1 // 2)) * CH

                # qW[c', m] = sum_c (scale*Wk)[c, c'] q[c, m]
                pq = psq.tile([128, 512], F32)
                nc.tensor.matmul(pq[:], lhsT=whh[:, wsl], rhs=qhh[:], start=True, stop=False)
                nc.tensor.matmul(pq[:], lhsT=whh[:, wsl], rhs=qll[:], start=False, stop=False)
                nc.tensor.matmul(pq[:], lhsT=whl[:, wsl], rhs=qhh[:], start=False, stop=True)
                nc.scalar.copy(qw[rows, cc2:cc2 + CH], pq[rows, :])

                # qp[j, m] = sum_c Wkp[c, j] q[c, m]
                pp = psp.tile([4, 512], F32)
                nc.tensor.matmul(pp[:], lhsT=wkph[:], rhs=qhh[:], start=True, stop=False)
                nc.tensor.matmul(pp[:], lhsT=wkph[:], rhs=qll[:], start=False, stop=False)
                nc.tensor.matmul(pp[:], lhsT=wkpl[:], rhs=qhh[:], start=False, stop=True)

                g, gi = divmod(t, 4)
                if g not in qps_tiles:
                    qps_tiles[g] = qpsp.tile([4, 2048], F32, name="qps", tag="qps")
                    qps_fill[g] = 0
                qps = qps_tiles[g]
                nc.scalar.copy(qps[:, gi * 512:(gi + 1) * 512], pp[:])
                qps_fill[g] += 1
                if qps_fill[g] == 4:
                    # scatter qp group into softmax layout: qpt[p, j*64+mi]
                    for j in range(4):
                        nc.gpsimd.dma_start(
                            qpt[g * 32:(g + 1) * 32, j * 64:(j + 1) * 64],
                            qps[j:j + 1, :],
                        )
                    del qps_tiles[g]

        # ---- phase 2: positional term (whole core at once) ----
        # tmp[p, mi, j, s] = xyz[j, m] - nx[j, m, s]
        nx3 = nxt[:].rearrange("p (mi j s) -> p mi j s", mi=64, j=3, s=S)
        xyzb = (
            xyzt[:]
            .rearrange("p (mi j one) -> p mi j one", mi=64, j=3, one=1)
            .broadcast_to((128, 64, 3, S))
        )
        tmp = p3072.tile([128, 64 * 3 * S], F32, tag="big")
        tmp3 = tmp[:].rearrange("p (mi j s) -> p mi j s", mi=64, j=3, s=S)
        nc.vector.tensor_sub(tmp3, xyzb, nx3)

        sq = p3072.tile([128, 64 * 3 * S], F32, tag="big")
        nc.scalar.square(sq[:], tmp[:])

        norm2 = p1024.tile([128, 64 * S], F32, tag="w1k")
        nc.vector.reduce_sum(
            norm2[:].rearrange("p (mi s) -> p mi s", mi=64),
            sq[:].rearrange("p (mi j s) -> p mi s j", mi=64, j=3, s=S),
            axis=AX.X,
        )
        norm = p1024.tile([128, 64 * S], F32, tag="w1k")
        nc.scalar.sqrt(norm[:], norm2[:])

        # u = sum_j qp[j]*tmp[j]
        qptb3 = (
            qpt[:]
            .rearrange("p (j mi one) -> p mi j one", j=4, mi=64, one=1)[:, :, 0:3, :]
            .broadcast_to((128, 64, 3, S))
        )
        uw = p3072.tile([128, 64 * 3 * S], F32, tag="big")
        uw3 = uw[:].rearrange("p (mi j s) -> p mi j s", mi=64, j=3, s=S)
        nc.vector.tensor_mul(uw3, tmp3, qptb3)
        u = p1024.tile([128, 64 * S], F32, tag="w1k")
        nc.vector.reduce_sum(
            u[:].rearrange("p (mi s) -> p mi s", mi=64),
            uw[:].rearrange("p (mi j s) -> p mi s j", mi=64, j=3, s=S),
            axis=AX.X,
        )

        # attn2 = u + qp3 * norm
        qp3b = (
            qpt[:, 192:256]
            .rearrange("p (mi one) -> p mi one", one=1)
            .broadcast_to((128, 64, S))
        )
        a2 = p1024.tile([128, 64 * S], F32, tag="w1k")
        a23 = a2[:].rearrange("p (mi s) -> p mi s", mi=64)
        nc.vector.tensor_mul(a23, norm[:].rearrange("p (mi s) -> p mi s", mi=64), qp3b)
        nc.vector.tensor_add(attn2[:], a2[:], u[:])

        # ---- phase 3: main term supertiles ----
        if True:
            for t in range(NT):
                npt = npp.tile([128, MB * S], F32)
                dma(npt[:], NP[:, t * MB * S:(t + 1) * MB * S])

                prod = prp.tile([128, MB * S], RDT)
                qwb = (
                    qw[:, t * MB:(t + 1) * MB]
                    .rearrange("p (mm one) -> p mm one", one=1)
                    .broadcast_to((128, MB, S))
                )
                nc.vector.tensor_mul(
                    prod[:].rearrange("p (mm s) -> p mm s", s=S),
                    npt[:].rearrange("p (mm s) -> p mm s", s=S),
                    qwb,
                )

                ps = psm.tile([16, 512], F32)
                for k in range(8):
                    nc.tensor.matmul(
                        ps[:],
                        lhsT=hs[:, k * 16:(k + 1) * 16],
                        rhs=prod[:, k * 512:(k + 1) * 512],
                        start=(k == 0),
                        stop=(k == 7),
                    )
                sc = scp.tile([16, 512], F32)
                nc.scalar.copy(sc[:], ps[:])
                # row h*8+k holds chunk k / half h; lands at attn1 partition
                # h*64 + t*4 + k//2, cols (k%2)*512 + i*16 + s. Issued from
                # GPSIMD (SWDGE) so their waits don't stall the Sync queue
                # that prefetches NP tiles.
                for h in range(2):
                    nc.gpsimd.dma_start(
                        attn1[h * 64 + t * 4:h * 64 + t * 4 + 4, :].rearrange(
                            "p (k1 f) -> p k1 f", k1=2
                        ),
                        sc[h * 8:(h + 1) * 8, :],
                    )

        # ---- phase 4: softmax over s ----
        attn = p1024.tile([128, 64 * S], F32, tag="w1k")
        nc.vector.tensor_add(attn[:], attn1[:], attn2[:])
        at3 = attn[:].rearrange("p (mi s) -> p mi s", mi=64)

        mx = sp.tile([128, 64], F32)
        nc.vector.reduce_max(mx[:], at3, axis=AX.X)
        mxb = mx[:].rearrange("p (mi one) -> p mi one", one=1).broadcast_to((128, 64, S))
        xs = p1024.tile([128, 64 * S], F32, tag="w1k")
        nc.vector.tensor_sub(xs[:].rearrange("p (mi s) -> p mi s", mi=64), at3, mxb)

        e = p1024.tile([128, 64 * S], F32, tag="w1k")
        nc.scalar.activation(e[:], xs[:], AF.Exp)

        se = sp.tile([128, 64], F32)
        nc.vector.reduce_sum(se[:], e[:].rearrange("p (mi s) -> p mi s", mi=64), axis=AX.X)
        rse = sp.tile([128, 64], F32)
        nc.vector.reciprocal(rse[:], se[:])

        o = p1024.tile([128, 64 * S], F32, tag="w1k")
        rb = rse[:].rearrange("p (mi one) -> p mi one", one=1).broadcast_to((128, 64, S))
        nc.vector.tensor_mul(
            o[:].rearrange("p (mi s) -> p mi s", mi=64),
            e[:].rearrange("p (mi s) -> p mi s", mi=64),
            rb,
        )
        dma(OUT, o[:])


_NC_CACHE = None


def build_nc():
    global _NC_CACHE
    if _NC_CACHE is None:
        nc = bacc.Bacc(trn_type="TRN2", target_bir_lowering=False, debug=False)
        with tile.TileContext(nc) as tc:
            _body(tc)
        nc.compile()
        _NC_CACHE = nc
    return _NC_CACHE


def make_in_maps(xyz, neighbor_xyz, points, neighbor_points, Wk, Wpos, bpos):
    """Slice + relayout full inputs into the 8 per-core input maps."""
    xyz = np.asarray(xyz, dtype=np.float32)
    neighbor_xyz = np.asarray(neighbor_xyz, dtype=np.float32)
    points = np.asarray(points, dtype=np.float32)
    neighbor_points = np.asarray(neighbor_points, dtype=np.float32)
    Wk = np.ascontiguousarray(np.asarray(Wk, dtype=np.float32))
    WkT = np.ascontiguousarray(Wk.T)
    Wp = np.ascontiguousarray(np.asarray(Wpos, dtype=np.float32))

    in_maps = []
    for i in range(NCORES):
        nsl = slice(i * NL, (i + 1) * NL)
        # np: [B,C,nl,S] -> [c, m, s] -> [h, c, mm, s] -> [128, MH*S]
        npc = neighbor_points[:, :, nsl, :].transpose(1, 0, 2, 3).reshape(C, M, S)
        npc = (
            npc.reshape(C, 2, MH, S).transpose(1, 0, 2, 3).reshape(128, MH * S)
        )
        # nx: [B,3,nl,S] -> [m, j, s] -> [128, 64*3*S]
        nxc = (
            neighbor_xyz[:, :, nsl, :]
            .transpose(1, 0, 2, 3)
            .reshape(3, M, S)
            .transpose(1, 0, 2)
            .reshape(128, 64 * 3 * S)
        )
        # xyz: [B,3,nl] -> [m, j] -> [128, 192]
        xc = (
            xyz[:, :, nsl]
            .transpose(1, 0, 2)
            .reshape(3, M)
            .T.reshape(128, 64 * 3)
        )
        # points: [B,C,nl] -> [c, m]
        pc = points[:, :, nsl].transpose(1, 0, 2).reshape(C, M)
        in_maps.append(
            {
                "NP": np.ascontiguousarray(npc),
                "NX": np.ascontiguousarray(nxc),
                "XYZ": np.ascontiguousarray(xc),
                "P": np.ascontiguousarray(pc),
                "WK": Wk,
                "WKT": WkT,
                "WP": Wp,
            }
        )
    return in_maps


def assemble_output(results):
    """Per-core OUT [128, 64*S] -> full [B, N, S]."""
    out = np.empty((B, N, S), dtype=np.float32)
    for i in range(NCORES):
        oc = np.asarray(results[i]["OUT"]).reshape(M, S)  # m = p*64+mi row-major
        out[:, i * NL:(i + 1) * NL, :] = oc.reshape(B, NL, S)
    return out


def run_cores(in_maps, trace=False, trace_kwargs=None):
    nc = build_nc()
    return run_bass_kernel_spmd(
        nc,
        in_maps,
        core_ids=list(range(NCORES)),
        trace=trace,
        **(trace_kwargs or {}),
    )


def kernel(xyz, neighbor_xyz, points, neighbor_points, Wk, Wpos, bpos):
    in_maps = make_in_maps(
        xyz, neighbor_xyz, points, neighbor_points, Wk, Wpos, bpos
    )
    res = run_cores(in_maps, trace=False)
    return assemble_output(res.results)



# revision 11
# speedup vs baseline: 1.7464x; 1.7464x over previous
"""Trainium2 Bass kernel for nn_AttentionScore (sparse local attention scores).

Reference computation (B=4, C=64, N=16384, S=16):
    tmp   = xyz[:, :, :, None] - neighbor_xyz            # [B,3,N,S]
    pos   = concat([tmp, ||tmp||], axis=1)               # [B,4,N,S]
    k     = Wk @ (neighbor_points + Wpos @ pos + bpos)   # [B,C,N,S]
    attn  = softmax_s((points*scale) . k)                # [B,N,S]

Softmax over s is shift-invariant, so every term constant in s drops out:
    attn[m,s] ~ sum_c qW[c,m]*np[c,m,s] + sum_j qp[j,m]*tmp[j,m,s] + qp3[m]*||tmp||
with qW[ci,m] = sum_co scale*Wk[co,ci]*points[co,m], qp = (scale*Wk@Wpos)^T @ points
(bpos and the xyz.qp dot cancel).

Sharding: N split contiguously across 8 cores (no communication needed).
m = b*2048 + n_local in [0, 8192) per core; halves h = m // 4096.

All bulk inputs are host-cast to bf16 (output tolerance is 2e-2; measured
error stays ~1e-3), halving HBM traffic. The main-term multiply uses an
(mq, s, mi) column order so the broadcast-qW operand has an innermost
unit-stride AP dim -> DVE 2x_1p bf16 mode. The channel reduction and the
positional j-reduction both run on TensorE with 0/1 selector matrices
shipped from the host; their PSUM rows land directly on softmax-layout
partitions so PSUM->SBUF copies never shift partitions.

Layouts per core (m = row index in [0, 8192), h = m//4096):
  NP  [128, 65536] bf16: part h*64+c, col t*8192 + mq*1024 + s*64 + mi
      where within half: mm = t*512 + mq*64 + mi   (8 supertiles)
  NX  [96, 4096]  bf16: part j*32 + (m//64)%32, col (m//2048)*1024 + s*64 + m%64
  XYZ [96, 256]   bf16: part j*32 + (m//64)%32, col (m//2048)*64 + m%64
  P   [64, 8192]  bf16: points[c, m]
  HS/SELN/SELA: 0/1 selector matrices (see make_in_maps)
  OUT [128, 1024] bf16: part m//64, col s*64 + m%64
"""

import sys

sys.path.insert(0, "/opt/trn_rl_repo")

import numpy as np
import ml_dtypes

import concourse.bass as bass
import concourse.bacc as bacc
import concourse.tile as tile
from concourse import mybir
from concourse.bass_utils import run_bass_kernel_spmd

F32 = mybir.dt.float32
BF16 = mybir.dt.bfloat16
AF = mybir.ActivationFunctionType
AX = mybir.AxisListType

BF = ml_dtypes.bfloat16

B, C, N, S = 4, 64, 16384, 16
NCORES = 8
NL = N // NCORES            # 2048 points per core
M = B * NL                  # 8192 (b, n) rows per core
MH = M // 2                 # 4096 rows per half
ST = 8                      # supertiles
MB = MH // ST               # 512 mm per supertile (per half)
SCALE = float(C) ** -0.5


DEBUG = False


def _body(tc):
    nc = tc.nc

    NP = nc.dram_tensor("NP", [128, ST * 8192], BF16, kind="ExternalInput").ap()
    NX = nc.dram_tensor("NX", [96, 4096], BF16, kind="ExternalInput").ap()
    XYZ = nc.dram_tensor("XYZ", [96, 256], BF16, kind="ExternalInput").ap()
    P = nc.dram_tensor("P", [C, M], BF16, kind="ExternalInput").ap()
    WK = nc.dram_tensor("WK", [C, C], F32, kind="ExternalInput").ap()
    WKT = nc.dram_tensor("WKT", [C, C], F32, kind="ExternalInput").ap()
    WP = nc.dram_tensor("WP", [C, 4], F32, kind="ExternalInput").ap()
    HS = nc.dram_tensor("HS", [128, 512], BF16, kind="ExternalInput").ap()
    SELN = nc.dram_tensor("SELN", [96, 128], BF16, kind="ExternalInput").ap()
    SELA = nc.dram_tensor("SELA", [128, 512], BF16, kind="ExternalInput").ap()
    OUT = nc.dram_tensor("OUT", [128, 1024], BF16, kind="ExternalOutput").ap()

    with (
        tc.tile_pool(name="const", bufs=1) as cp,
        tc.tile_pool(name="small", bufs=1) as sp,

        tc.tile_pool(name="tmp4", bufs=4) as tmpp,
        tc.tile_pool(name="sq2", bufs=2) as sqp,
        tc.tile_pool(name="pp2", bufs=2) as ppp,
        tc.tile_pool(name="npt", bufs=4) as npp,
        tc.tile_pool(name="prod", bufs=2) as prp,
        tc.tile_pool(name="sc", bufs=2) as scp,
        tc.tile_pool(name="soft", bufs=1) as smp,
        tc.tile_pool(name="psq", bufs=2, space="PSUM") as psq,
        tc.tile_pool(name="psn", bufs=2, space="PSUM") as psn,
        tc.tile_pool(name="psa", bufs=1, space="PSUM") as psa,
        tc.tile_pool(name="psm", bufs=3, space="PSUM") as psm,
    ):
        # ---- NP stream: sole occupant of the Sync HWDGE queue ----
        npts = []
        for t in range(ST):
            npt = npp.tile([128, 8192], BF16, name=f"np{t}", tag="np")
            nc.sync.dma_start(npt[:], NP[:, t * 8192:(t + 1) * 8192])
            npts.append(npt)

        # ---- constants / bulk side inputs on the Activation HWDGE queue ----
        pt = cp.tile([C, M], BF16)
        nc.scalar.dma_start(pt[:], P)
        nxt = cp.tile([96, 4096], BF16)
        nc.scalar.dma_start(nxt[:], NX)
        xyzt = cp.tile([96, 256], BF16)
        nc.scalar.dma_start(xyzt[:], XYZ)
        wk = cp.tile([C, C], F32)
        nc.scalar.dma_start(wk[:], WK)
        wkt = cp.tile([C, C], F32)
        nc.scalar.dma_start(wkt[:], WKT)
        wp = cp.tile([C, 4], F32)
        nc.scalar.dma_start(wp[:], WP)
        hs = cp.tile([128, 512], BF16)
        nc.scalar.dma_start(hs[:], HS)
        seln = cp.tile([96, 128], BF16)
        nc.scalar.dma_start(seln[:], SELN)
        sela = cp.tile([128, 512], BF16)
        nc.scalar.dma_start(sela[:], SELA)

        qw = cp.tile([128, MH], BF16)      # row h*64+c: qW[c, h*MH + mm]
        qpt2 = cp.tile([128, 256], BF16)   # row j*32+mbq: qp[j, (t4*32+mbq)*64+mi]
        attn1 = cp.tile([128, 1024], F32)  # part m//64, col s*64 + m%64
        attn2 = cp.tile([128, 1024], F32)

        # ---- phase 1: qW / qp, one bf16 matmul per 512-m chunk ----
        # wl_h [64, 128]: cols h*64..+64 = scale*Wk (-> qW rows), cols
        # (1-h)*64..+4 = scale*Wk@Wpos (-> qp rows). Both output row groups
        # land on the partitions their consumers read, so the PSUM->SBUF
        # copies are partition-aligned.
        wks = sp.tile([C, C], F32)
        nc.vector.tensor_scalar_mul(wks[:], wk[:], SCALE)
        wkts = sp.tile([C, C], F32)
        nc.vector.tensor_scalar_mul(wkts[:], wkt[:], SCALE)
        pwkp = psa.tile([C, 4], F32, tag="pa")
        nc.tensor.matmul(pwkp[:], lhsT=wkts[:], rhs=wp[:], start=True, stop=True)
        wkp = sp.tile([C, 4], F32)
        nc.scalar.copy(wkp[:], pwkp[:])

        wl = []
        for h in (0, 1):
            wlf = sp.tile([C, 128], F32, name=f"wlf{h}")
            nc.vector.memset(wlf[:], 0.0)
            nc.vector.tensor_copy(wlf[:, h * 64:(h + 1) * 64], wks[:])
            nc.vector.tensor_copy(wlf[:, (1 - h) * 64:(1 - h) * 64 + 4], wkp[:])
            wlb = sp.tile([C, 128], BF16, name=f"wl{h}")
            nc.vector.tensor_copy(wlb[:], wlf[:])
            wl.append(wlb)

        # qp staging: h=1 qp rows land on partitions 0..4, h=0 on 64..68;
        # columns are m_loc within the half. Scattered to qpt2 afterwards.
        qps = cp.tile([68, MH], BF16)
        for cc in range(8):
            for h in range(2):
                pq = psq.tile([128, 512], F32)
                csl = slice(h * MH + cc * 512, h * MH + (cc + 1) * 512)
                nc.tensor.matmul(pq[:], lhsT=wl[h][:], rhs=pt[:, csl], start=True, stop=True)
                nc.scalar.copy(qw[h * 64:(h + 1) * 64, cc * 512:(cc + 1) * 512],
                               pq[h * 64:(h + 1) * 64, :])
                r0 = (1 - h) * 64
                nc.scalar.copy(qps[r0:r0 + 4, cc * 512:(cc + 1) * 512],
                               pq[r0:r0 + 4, :])
        # scatter: qpt2[j*32 + b%32, (2h + b//32)*64 + mi] = qp[j, h*MH + b*64 + mi]
        for h in range(2):
            for j in range(4):
                for b2 in range(2):
                    t4 = 2 * h + b2
                    nc.gpsimd.dma_start(
                        qpt2[j * 32:(j + 1) * 32, t4 * 64:(t4 + 1) * 64],
                        qps[(1 - h) * 64 + j:(1 - h) * 64 + j + 1,
                            b2 * 2048:(b2 + 1) * 2048]
                        .rearrange("p (q mi) -> p q mi", q=32, mi=64),
                    )

        # ---- phase 2a: tmp = xyz - nx, norm via TensorE selector reduce ----
        # tmp4[t4] rows: j*32+mbq = tmp_j for m-block t4*32+mbq; rows 96..128
        # get ||tmp|| (sqrt writes straight from PSUM rows 96..128).
        tmps = []
        for t4 in range(4):
            csl = slice(t4 * 1024, (t4 + 1) * 1024)
            tmp = tmpp.tile([128, 1024], BF16, name=f"tmp{t4}", tag="tmp")
            nc.vector.tensor_sub(
                tmp[0:96, :].rearrange("p (s mi) -> p s mi", s=S),
                xyzt[:, t4 * 64:(t4 + 1) * 64]
                .rearrange("p (one mi) -> p one mi", one=1)
                .broadcast_to((96, S, 64)),
                nxt[:, csl].rearrange("p (s mi) -> p s mi", s=S),
            )
            sq = sqp.tile([96, 1024], BF16)
            nc.scalar.square(sq[:], tmp[0:96, :])
            for half in range(2):
                pn = psn.tile([128, 512], F32)
                nc.tensor.matmul(pn[:], lhsT=seln[:],
                                 rhs=sq[:, half * 512:(half + 1) * 512],
                                 start=True, stop=True)
                nc.scalar.sqrt(tmp[96:128, half * 512:(half + 1) * 512],
                               pn[96:128, :])
            tmps.append(tmp)

        # ---- phase 3: main term supertiles (+ interleaved pos term) ----
        for t in range(ST):
            npt = npts[t]
            prod = prp.tile([128, 8192], BF16)
            qwb = (
                qw[:, t * 512:(t + 1) * 512]
                .rearrange("p (mq one mi) -> p mq one mi", mq=8, one=1, mi=64)
                .broadcast_to((128, 8, S, 64))
            )
            nc.vector.tensor_mul(
                prod[:].rearrange("p (mq s mi) -> p mq s mi", mq=8, s=S),
                npt[:].rearrange("p (mq s mi) -> p mq s mi", mq=8, s=S),
                qwb,
            )
            ps = psm.tile([32, 512], F32)
            for k in range(16):
                nc.tensor.matmul(
                    ps[:],
                    lhsT=hs[:, k * 32:(k + 1) * 32],
                    rhs=prod[:, k * 512:(k + 1) * 512],
                    start=(k == 0),
                    stop=(k == 15),
                )
            sc = scp.tile([32, 512], F32)
            nc.scalar.copy(sc[:], ps[:])
            # row h*16+k holds (mq=k//2, s-half=k%2) -> partition h*64+t*8+k//2,
            # col (k%2)*512 + si*64 + mi: 2KB-contiguous per row.
            for h in range(2):
                nc.gpsimd.dma_start(
                    attn1[h * 64 + t * 8:h * 64 + t * 8 + 8, :]
                    .rearrange("p (k1 f) -> p k1 f", k1=2),
                    sc[h * 16:(h + 1) * 16, :],
                )

            # phase 2b interleaved: pos products + TensorE j-reduce for one
            # m-quarter. Fills the DVE/PE gaps while NP streams.
            if t < 4:
                t4 = t
                tmp = tmps[t4]
                pp = ppp.tile([128, 1024], BF16)
                nc.vector.tensor_mul(
                    pp[:].rearrange("p (s mi) -> p s mi", s=S),
                    tmp[:].rearrange("p (s mi) -> p s mi", s=S),
                    qpt2[:, t4 * 64:(t4 + 1) * 64]
                    .rearrange("p (one mi) -> p one mi", one=1)
                    .broadcast_to((128, S, 64)),
                )
                for half in range(2):
                    pa = psa.tile([128, 512], F32, tag="pa")
                    nc.tensor.matmul(pa[:], lhsT=sela[:, t4 * 128:(t4 + 1) * 128],
                                     rhs=pp[:, half * 512:(half + 1) * 512],
                                     start=True, stop=True)
                    nc.scalar.copy(
                        attn2[t4 * 32:(t4 + 1) * 32, half * 512:(half + 1) * 512],
                        pa[t4 * 32:(t4 + 1) * 32, :])

        # ---- phase 4: softmax over s (|attn| < ~6, so no max-subtract) ----
        att = smp.tile([128, 1024], F32)
        nc.vector.tensor_add(att[:], attn1[:], attn2[:])
        e = smp.tile([128, 1024], BF16)
        nc.scalar.activation(e[:], att[:], AF.Exp)
        se = smp.tile([128, 64], F32)
        nc.vector.reduce_sum(
            se[:], e[:].rearrange("p (s mi) -> p mi s", s=S), axis=AX.X
        )
        rse = smp.tile([128, 64], F32)
        nc.vector.reciprocal(rse[:], se[:])
        rse16 = smp.tile([128, 64], BF16)
        nc.vector.tensor_copy(rse16[:], rse[:])
        o = smp.tile([128, 1024], BF16)
        nc.vector.tensor_mul(
            o[:].rearrange("p (s mi) -> p s mi", s=S),
            e[:].rearrange("p (s mi) -> p s mi", s=S),
            rse16[:].rearrange("p (one mi) -> p one mi", one=1)
            .broadcast_to((128, S, 64)),
        )
        nc.scalar.dma_start(OUT, o[:])

        if DEBUG:
            DQW = nc.dram_tensor("DQW", [128, MH], BF16, kind="ExternalOutput").ap()
            DQP = nc.dram_tensor("DQP", [128, 256], BF16, kind="ExternalOutput").ap()
            DA1 = nc.dram_tensor("DA1", [128, 1024], F32, kind="ExternalOutput").ap()
            DA2 = nc.dram_tensor("DA2", [128, 1024], F32, kind="ExternalOutput").ap()
            DT0 = nc.dram_tensor("DT0", [128, 1024], BF16, kind="ExternalOutput").ap()
            nc.scalar.dma_start(DQW, qw[:])
            nc.scalar.dma_start(DQP, qpt2[:])
            nc.scalar.dma_start(DA1, attn1[:])
            nc.scalar.dma_start(DA2, attn2[:])
            nc.scalar.dma_start(DT0, tmps[0][:])


_NC_CACHE = None


def build_nc():
    global _NC_CACHE
    if _NC_CACHE is None:
        nc = bacc.Bacc(trn_type="TRN2", target_bir_lowering=False, debug=False)
        with tile.TileContext(nc) as tc:
            _body(tc)
        nc.compile()
        _NC_CACHE = nc
    return _NC_CACHE


def _selectors():
    # HS [128, 512]: hs[p, k*32 + r] = 1 iff r == (p//64)*16 + k
    hs = np.zeros((128, 512), dtype=BF)
    for k in range(16):
        for h in range(2):
            hs[h * 64:(h + 1) * 64, k * 32 + h * 16 + k] = 1
    # SELN [96, 128]: col 96+q sums partitions {q, 32+q, 64+q}
    seln = np.zeros((96, 128), dtype=BF)
    for q in range(32):
        for j in range(3):
            seln[j * 32 + q, 96 + q] = 1
    # SELA [128, 512]: col t4*128 + r (r in [t4*32, t4*32+32)) sums
    # partitions {r%32 + 32j : j in 0..4}
    sela = np.zeros((128, 512), dtype=BF)
    for t4 in range(4):
        for q in range(32):
            for j in range(4):
                sela[j * 32 + q, t4 * 128 + t4 * 32 + q] = 1
    return hs, seln, sela


def make_in_maps(xyz, neighbor_xyz, points, neighbor_points, Wk, Wpos, bpos):
    """Slice + relayout full inputs into the 8 per-core input maps."""
    xyz = np.asarray(xyz, dtype=np.float32)
    neighbor_xyz = np.asarray(neighbor_xyz, dtype=np.float32)
    points = np.asarray(points, dtype=np.float32)
    neighbor_points = np.asarray(neighbor_points, dtype=np.float32)
    Wk = np.ascontiguousarray(np.asarray(Wk, dtype=np.float32))
    WkT = np.ascontiguousarray(Wk.T)
    Wp = np.ascontiguousarray(np.asarray(Wpos, dtype=np.float32))
    hs, seln, sela = _selectors()

    in_maps = []
    for i in range(NCORES):
        nsl = slice(i * NL, (i + 1) * NL)
        # np: [B,C,nl,S] -> [c, m, s] -> (h, c, t, mq, s, mi) -> [128, 65536]
        npc = neighbor_points[:, :, nsl, :].transpose(1, 0, 2, 3).reshape(C, M, S)
        npc = (
            npc.reshape(C, 2, ST, 8, 64, S)
            .transpose(1, 0, 2, 3, 5, 4)
            .reshape(128, ST * 8192)
        )
        # nx: [B,3,nl,S] -> [j, m, s] -> (j, mbq, t4, s, mi) -> [96, 4096]
        nxc = (
            neighbor_xyz[:, :, nsl, :]
            .transpose(1, 0, 2, 3)
            .reshape(3, M, S)
            .reshape(3, 4, 32, 64, S)
            .transpose(0, 2, 1, 4, 3)
            .reshape(96, 4096)
        )
        # xyz: [B,3,nl] -> (j, mbq, t4, mi) -> [96, 256]
        xc = (
            xyz[:, :, nsl]
            .transpose(1, 0, 2)
            .reshape(3, M)
            .reshape(3, 4, 32, 64)
            .transpose(0, 2, 1, 3)
            .reshape(96, 256)
        )
        # points: [B,C,nl] -> [c, m]
        pc = points[:, :, nsl].transpose(1, 0, 2).reshape(C, M)
        in_maps.append(
            {
                "NP": np.ascontiguousarray(npc.astype(BF)),
                "NX": np.ascontiguousarray(nxc.astype(BF)),
                "XYZ": np.ascontiguousarray(xc.astype(BF)),
                "P": np.ascontiguousarray(pc.astype(BF)),
                "WK": Wk,
                "WKT": WkT,
                "WP": Wp,
                "HS": hs,
                "SELN": seln,
                "SELA": sela,
            }
        )
    return in_maps


def assemble_output(results):
    """Per-core OUT [128, 1024] bf16 (p, s, mi) -> full [B, N, S] f32."""
    out = np.empty((B, N, S), dtype=np.float32)
    for i in range(NCORES):
        oc = np.asarray(results[i]["OUT"]).astype(np.float32)
        oc = oc.reshape(128, S, 64).transpose(0, 2, 1).reshape(M, S)
        out[:, i * NL:(i + 1) * NL, :] = oc.reshape(B, NL, S)
    return out


def run_cores(in_maps, trace=False, trace_kwargs=None):
    nc = build_nc()
    return run_bass_kernel_spmd(
        nc,
        in_maps,
        core_ids=list(range(NCORES)),
        trace=trace,
        **(trace_kwargs or {}),
    )


def kernel(xyz, neighbor_xyz, points, neighbor_points, Wk, Wpos, bpos):
    in_maps = make_in_maps(
        xyz, neighbor_xyz, points, neighbor_points, Wk, Wpos, bpos
    )
    res = run_cores(in_maps, trace=False)
    return assemble_output(res.results)


# revision 15
# speedup vs baseline: 1.8316x; 1.0488x over previous
"""Trainium2 Bass kernel for nn_AttentionScore (sparse local attention scores).

Reference computation (B=4, C=64, N=16384, S=16):
    tmp   = xyz[:, :, :, None] - neighbor_xyz            # [B,3,N,S]
    pos   = concat([tmp, ||tmp||], axis=1)               # [B,4,N,S]
    k     = Wk @ (neighbor_points + Wpos @ pos + bpos)   # [B,C,N,S]
    attn  = softmax_s((points*scale) . k)                # [B,N,S]

Softmax over s is shift-invariant, so every term constant in s drops out:
    attn[m,s] ~ sum_c qW[c,m]*np[c,m,s] + sum_j qp[j,m]*tmp[j,m,s] + qp3[m]*||tmp||
with qW[ci,m] = sum_co scale*Wk[co,ci]*points[co,m], qp = (scale*Wk@Wpos)^T @ points
(bpos and the xyz.qp dot cancel).

Sharding: N split contiguously across 8 cores (no communication needed).
m = b*2048 + n_local in [0, 8192) per core; halves h = m // 4096.

All bulk inputs are host-cast to bf16 (output tolerance is 2e-2; measured
error stays ~1e-3), halving HBM traffic. The main-term multiply uses an
(mq, s, mi) column order so the broadcast-qW operand has an innermost
unit-stride AP dim -> DVE 2x_1p bf16 mode. The channel reduction and the
positional j-reduction both run on TensorE with 0/1 selector matrices
shipped from the host; their PSUM rows land directly on softmax-layout
partitions so PSUM->SBUF copies never shift partitions.

Layouts per core (m = row index in [0, 8192), h = m//4096):
  NP  [128, 65536] bf16: part h*64+c, col t*8192 + mq*1024 + s*64 + mi
      where within half: mm = t*512 + mq*64 + mi   (8 supertiles)
  NX  [96, 4096]  bf16: part j*32 + (m//64)%32, col (m//2048)*1024 + s*64 + m%64
  XYZ [96, 256]   bf16: part j*32 + (m//64)%32, col (m//2048)*64 + m%64
  P   [64, 8192]  bf16: points[c, m]
  HS/SELN/SELA: 0/1 selector matrices (see make_in_maps)
  OUT [128, 1024] bf16: part m//64, col s*64 + m%64
"""

import sys

sys.path.insert(0, "/opt/trn_rl_repo")

import numpy as np
import ml_dtypes

import concourse.bass as bass
import concourse.bacc as bacc
import concourse.tile as tile
from concourse import mybir
from concourse.bass_utils import run_bass_kernel_spmd

F32 = mybir.dt.float32
BF16 = mybir.dt.bfloat16
AF = mybir.ActivationFunctionType
AX = mybir.AxisListType

BF = ml_dtypes.bfloat16

B, C, N, S = 4, 64, 16384, 16
NCORES = 8
NL = N // NCORES            # 2048 points per core
M = B * NL                  # 8192 (b, n) rows per core
MH = M // 2                 # 4096 rows per half
ST = 8                      # supertiles
MB = MH // ST               # 512 mm per supertile (per half)
SCALE = float(C) ** -0.5


DEBUG = False


def _body(tc):
    nc = tc.nc

    NP = nc.dram_tensor("NP", [128, ST * 8192], BF16, kind="ExternalInput").ap()
    NX = nc.dram_tensor("NX", [96, 4096], BF16, kind="ExternalInput").ap()
    XYZ = nc.dram_tensor("XYZ", [96, 256], BF16, kind="ExternalInput").ap()
    P = nc.dram_tensor("P", [C, M], BF16, kind="ExternalInput").ap()
    WK = nc.dram_tensor("WK", [C, C], F32, kind="ExternalInput").ap()
    WKT = nc.dram_tensor("WKT", [C, C], F32, kind="ExternalInput").ap()
    WP = nc.dram_tensor("WP", [C, 4], F32, kind="ExternalInput").ap()
    HS = nc.dram_tensor("HS", [128, 512], BF16, kind="ExternalInput").ap()
    SELN = nc.dram_tensor("SELN", [96, 128], BF16, kind="ExternalInput").ap()
    SELA = nc.dram_tensor("SELA", [128, 512], BF16, kind="ExternalInput").ap()
    OUT = nc.dram_tensor("OUT", [128, 1024], BF16, kind="ExternalOutput").ap()

    with (
        tc.tile_pool(name="const", bufs=1) as cp,
        tc.tile_pool(name="small", bufs=1) as sp,

        tc.tile_pool(name="tmp4", bufs=4) as tmpp,
        tc.tile_pool(name="sq2", bufs=2) as sqp,
        tc.tile_pool(name="pp2", bufs=2) as ppp,
        tc.tile_pool(name="npt", bufs=5) as npp,
        tc.tile_pool(name="prod", bufs=2) as prp,
        tc.tile_pool(name="sc", bufs=2) as scp,
        tc.tile_pool(name="soft", bufs=1) as smp,
        tc.tile_pool(name="psq", bufs=2, space="PSUM") as psq,
        tc.tile_pool(name="psn", bufs=2, space="PSUM") as psn,
        tc.tile_pool(name="psa", bufs=1, space="PSUM") as psa,
        tc.tile_pool(name="psm", bufs=3, space="PSUM") as psm,
    ):
        # small constants first on the scalar queue, ahead of the odd NP tiles
        xyzt = cp.tile([96, 256], BF16)
        nc.scalar.dma_start(xyzt[:], XYZ)
        wk = cp.tile([C, C], F32)
        nc.scalar.dma_start(wk[:], WK)
        wkt = cp.tile([C, C], F32)
        nc.scalar.dma_start(wkt[:], WKT)
        wp = cp.tile([C, 4], F32)
        nc.scalar.dma_start(wp[:], WP)
        hs = cp.tile([128, 512], BF16)
        nc.scalar.dma_start(hs[:], HS)
        seln = cp.tile([96, 128], BF16)
        nc.scalar.dma_start(seln[:], SELN)
        sela = cp.tile([128, 512], BF16)
        nc.scalar.dma_start(sela[:], SELA)
        # bulky side inputs ride SWDGE so they don't head-of-line block NP
        pt = cp.tile([C, M], BF16)
        nc.gpsimd.dma_start(pt[:], P)
        nxt = cp.tile([96, 4096], BF16)
        nc.gpsimd.dma_start(nxt[:], NX)

        # ---- NP stream striped across BOTH HWDGE queues: the SDMA engines
        # round-robin between the two rings at packet granularity, doubling
        # outstanding descriptors and hiding per-descriptor HBM latency.
        npts = []
        for t in range(ST):
            npt = npp.tile([128, 8192], BF16, name=f"np{t}", tag="np")
            eng = nc.sync if t % 2 == 0 else nc.scalar
            eng.dma_start(npt[:], NP[:, t * 8192:(t + 1) * 8192])
            npts.append(npt)

        qw = cp.tile([128, MH], BF16)      # row h*64+c: qW[c, h*MH + mm]
        qpt2 = cp.tile([128, 256], BF16)   # row j*32+mbq: qp[j, (t4*32+mbq)*64+mi]
        attn1 = cp.tile([128, 1024], F32)  # part m//64, col s*64 + m%64
        attn2 = cp.tile([128, 1024], F32)

        # ---- phase 1: qW / qp, one bf16 matmul per 512-m chunk ----
        # wl_h [64, 128]: cols h*64..+64 = scale*Wk (-> qW rows), cols
        # (1-h)*64..+4 = scale*Wk@Wpos (-> qp rows). Both output row groups
        # land on the partitions their consumers read, so the PSUM->SBUF
        # copies are partition-aligned.
        wks = sp.tile([C, C], F32)
        nc.vector.tensor_scalar_mul(wks[:], wk[:], SCALE)
        wkts = sp.tile([C, C], F32)
        nc.vector.tensor_scalar_mul(wkts[:], wkt[:], SCALE)
        pwkp = psa.tile([C, 4], F32, tag="pa")
        nc.tensor.matmul(pwkp[:], lhsT=wkts[:], rhs=wp[:], start=True, stop=True)
        wkp = sp.tile([C, 4], F32)
        nc.scalar.copy(wkp[:], pwkp[:])

        wl = []
        for h in (0, 1):
            wlf = sp.tile([C, 128], F32, name=f"wlf{h}")
            nc.vector.memset(wlf[:], 0.0)
            nc.vector.tensor_copy(wlf[:, h * 64:(h + 1) * 64], wks[:])
            nc.vector.tensor_copy(wlf[:, (1 - h) * 64:(1 - h) * 64 + 4], wkp[:])
            wlb = sp.tile([C, 128], BF16, name=f"wl{h}")
            nc.vector.tensor_copy(wlb[:], wlf[:])
            wl.append(wlb)

        # qp staging: h=1 qp rows land on partitions 0..4, h=0 on 64..68;
        # columns are m_loc within the half. Scattered to qpt2 afterwards.
        qps = cp.tile([68, MH], BF16)
        for cc in range(8):
            for h in range(2):
                pq = psq.tile([128, 512], F32)
                csl = slice(h * MH + cc * 512, h * MH + (cc + 1) * 512)
                nc.tensor.matmul(pq[:], lhsT=wl[h][:], rhs=pt[:, csl], start=True, stop=True)
                nc.scalar.copy(qw[h * 64:(h + 1) * 64, cc * 512:(cc + 1) * 512],
                               pq[h * 64:(h + 1) * 64, :])
                r0 = (1 - h) * 64
                nc.scalar.copy(qps[r0:r0 + 4, cc * 512:(cc + 1) * 512],
                               pq[r0:r0 + 4, :])
        # scatter: qpt2[j*32 + b%32, (2h + b//32)*64 + mi] = qp[j, h*MH + b*64 + mi]
        for h in range(2):
            for j in range(4):
                for b2 in range(2):
                    t4 = 2 * h + b2
                    nc.gpsimd.dma_start(
                        qpt2[j * 32:(j + 1) * 32, t4 * 64:(t4 + 1) * 64],
                        qps[(1 - h) * 64 + j:(1 - h) * 64 + j + 1,
                            b2 * 2048:(b2 + 1) * 2048]
                        .rearrange("p (q mi) -> p q mi", q=32, mi=64),
                    )

        # ---- phase 2a: tmp = xyz - nx, norm via TensorE selector reduce ----
        # tmp4[t4] rows: j*32+mbq = tmp_j for m-block t4*32+mbq; rows 96..128
        # get ||tmp|| (sqrt writes straight from PSUM rows 96..128).
        tmps = []
        for t4 in range(4):
            csl = slice(t4 * 1024, (t4 + 1) * 1024)
            tmp = tmpp.tile([128, 1024], BF16, name=f"tmp{t4}", tag="tmp")
            nc.vector.tensor_sub(
                tmp[0:96, :].rearrange("p (s mi) -> p s mi", s=S),
                xyzt[:, t4 * 64:(t4 + 1) * 64]
                .rearrange("p (one mi) -> p one mi", one=1)
                .broadcast_to((96, S, 64)),
                nxt[:, csl].rearrange("p (s mi) -> p s mi", s=S),
            )
            sq = sqp.tile([96, 1024], BF16)
            nc.scalar.square(sq[:], tmp[0:96, :])
            for half in range(2):
                pn = psn.tile([128, 512], F32)
                nc.tensor.matmul(pn[:], lhsT=seln[:],
                                 rhs=sq[:, half * 512:(half + 1) * 512],
                                 start=True, stop=True)
                nc.scalar.sqrt(tmp[96:128, half * 512:(half + 1) * 512],
                               pn[96:128, :])
            tmps.append(tmp)

        # ---- phase 3: main term supertiles (+ interleaved pos term) ----
        for t in range(ST):
            npt = npts[t]
            prod = prp.tile([128, 8192], BF16)
            qwb = (
                qw[:, t * 512:(t + 1) * 512]
                .rearrange("p (mq one mi) -> p mq one mi", mq=8, one=1, mi=64)
                .broadcast_to((128, 8, S, 64))
            )
            nc.vector.tensor_mul(
                prod[:].rearrange("p (mq s mi) -> p mq s mi", mq=8, s=S),
                npt[:].rearrange("p (mq s mi) -> p mq s mi", mq=8, s=S),
                qwb,
            )
            ps = psm.tile([32, 512], F32)
            for k in range(16):
                nc.tensor.matmul(
                    ps[:],
                    lhsT=hs[:, k * 32:(k + 1) * 32],
                    rhs=prod[:, k * 512:(k + 1) * 512],
                    start=(k == 0),
                    stop=(k == 15),
                )
            sc = scp.tile([32, 512], F32)
            nc.scalar.copy(sc[:], ps[:])
            # row h*16+k holds (mq=k//2, s-half=k%2) -> partition h*64+t*8+k//2,
            # col (k%2)*512 + si*64 + mi: 2KB-contiguous per row.
            seng = nc.scalar if t % 2 == 0 else nc.sync
            for h in range(2):
                seng.dma_start(
                    attn1[h * 64 + t * 8:h * 64 + t * 8 + 8, :]
                    .rearrange("p (k1 f) -> p k1 f", k1=2),
                    sc[h * 16:(h + 1) * 16, :],
                )

            # phase 2b interleaved: pos products + TensorE j-reduce for one
            # m-quarter. Fills the DVE/PE gaps while NP streams.
            if t < 4:
                t4 = t
                tmp = tmps[t4]
                pp = ppp.tile([128, 1024], BF16)
                nc.vector.tensor_mul(
                    pp[:].rearrange("p (s mi) -> p s mi", s=S),
                    tmp[:].rearrange("p (s mi) -> p s mi", s=S),
                    qpt2[:, t4 * 64:(t4 + 1) * 64]
                    .rearrange("p (one mi) -> p one mi", one=1)
                    .broadcast_to((128, S, 64)),
                )
                for half in range(2):
                    pa = psa.tile([128, 512], F32, tag="pa")
                    nc.tensor.matmul(pa[:], lhsT=sela[:, t4 * 128:(t4 + 1) * 128],
                                     rhs=pp[:, half * 512:(half + 1) * 512],
                                     start=True, stop=True)
                    nc.scalar.copy(
                        attn2[t4 * 32:(t4 + 1) * 32, half * 512:(half + 1) * 512],
                        pa[t4 * 32:(t4 + 1) * 32, :])

        # ---- phase 4: softmax over s (|attn| < ~6, so no max-subtract) ----
        att = smp.tile([128, 1024], F32)
        nc.vector.tensor_add(att[:], attn1[:], attn2[:])
        e = smp.tile([128, 1024], BF16)
        nc.scalar.activation(e[:], att[:], AF.Exp)
        se = smp.tile([128, 64], F32)
        nc.vector.reduce_sum(
            se[:], e[:].rearrange("p (s mi) -> p mi s", s=S), axis=AX.X
        )
        rse = smp.tile([128, 64], F32)
        nc.vector.reciprocal(rse[:], se[:])
        rse16 = smp.tile([128, 64], BF16)
        nc.vector.tensor_copy(rse16[:], rse[:])
        o = smp.tile([128, 1024], BF16)
        nc.vector.tensor_mul(
            o[:].rearrange("p (s mi) -> p s mi", s=S),
            e[:].rearrange("p (s mi) -> p s mi", s=S),
            rse16[:].rearrange("p (one mi) -> p one mi", one=1)
            .broadcast_to((128, S, 64)),
        )
        nc.scalar.dma_start(OUT, o[:])

        if DEBUG:
            DQW = nc.dram_tensor("DQW", [128, MH], BF16, kind="ExternalOutput").ap()
            DQP = nc.dram_tensor("DQP", [128, 256], BF16, kind="ExternalOutput").ap()
            DA1 = nc.dram_tensor("DA1", [128, 1024], F32, kind="ExternalOutput").ap()
            DA2 = nc.dram_tensor("DA2", [128, 1024], F32, kind="ExternalOutput").ap()
            DT0 = nc.dram_tensor("DT0", [128, 1024], BF16, kind="ExternalOutput").ap()
            nc.scalar.dma_start(DQW, qw[:])
            nc.scalar.dma_start(DQP, qpt2[:])
            nc.scalar.dma_start(DA1, attn1[:])
            nc.scalar.dma_start(DA2, attn2[:])
            nc.scalar.dma_start(DT0, tmps[0][:])


_NC_CACHE = None


def build_nc():
    global _NC_CACHE
    if _NC_CACHE is None:
        nc = bacc.Bacc(trn_type="TRN2", target_bir_lowering=False, debug=False)
        with tile.TileContext(nc) as tc:
            _body(tc)
        nc.compile()
        _NC_CACHE = nc
    return _NC_CACHE


def _selectors():
    # HS [128, 512]: hs[p, k*32 + r] = 1 iff r == (p//64)*16 + k
    hs = np.zeros((128, 512), dtype=BF)
    for k in range(16):
        for h in range(2):
            hs[h * 64:(h + 1) * 64, k * 32 + h * 16 + k] = 1
    # SELN [96, 128]: col 96+q sums partitions {q, 32+q, 64+q}
    seln = np.zeros((96, 128), dtype=BF)
    for q in range(32):
        for j in range(3):
            seln[j * 32 + q, 96 + q] = 1
    # SELA [128, 512]: col t4*128 + r (r in [t4*32, t4*32+32)) sums
    # partitions {r%32 + 32j : j in 0..4}
    sela = np.zeros((128, 512), dtype=BF)
    for t4 in range(4):
        for q in range(32):
            for j in range(4):
                sela[j * 32 + q, t4 * 128 + t4 * 32 + q] = 1
    return hs, seln, sela


def make_in_maps(xyz, neighbor_xyz, points, neighbor_points, Wk, Wpos, bpos):
    """Slice + relayout full inputs into the 8 per-core input maps."""
    xyz = np.asarray(xyz, dtype=np.float32)
    neighbor_xyz = np.asarray(neighbor_xyz, dtype=np.float32)
    points = np.asarray(points, dtype=np.float32)
    neighbor_points = np.asarray(neighbor_points, dtype=np.float32)
    Wk = np.ascontiguousarray(np.asarray(Wk, dtype=np.float32))
    WkT = np.ascontiguousarray(Wk.T)
    Wp = np.ascontiguousarray(np.asarray(Wpos, dtype=np.float32))
    hs, seln, sela = _selectors()

    in_maps = []
    for i in range(NCORES):
        nsl = slice(i * NL, (i + 1) * NL)
        # np: [B,C,nl,S] -> [c, m, s] -> (h, c, t, mq, s, mi) -> [128, 65536]
        npc = neighbor_points[:, :, nsl, :].transpose(1, 0, 2, 3).reshape(C, M, S)
        npc = (
            npc.reshape(C, 2, ST, 8, 64, S)
            .transpose(1, 0, 2, 3, 5, 4)
            .reshape(128, ST * 8192)
        )
        # nx: [B,3,nl,S] -> [j, m, s] -> (j, mbq, t4, s, mi) -> [96, 4096]
        nxc = (
            neighbor_xyz[:, :, nsl, :]
            .transpose(1, 0, 2, 3)
            .reshape(3, M, S)
            .reshape(3, 4, 32, 64, S)
            .transpose(0, 2, 1, 4, 3)
            .reshape(96, 4096)
        )
        # xyz: [B,3,nl] -> (j, mbq, t4, mi) -> [96, 256]
        xc = (
            xyz[:, :, nsl]
            .transpose(1, 0, 2)
            .reshape(3, M)
            .reshape(3, 4, 32, 64)
            .transpose(0, 2, 1, 3)
            .reshape(96, 256)
        )
        # points: [B,C,nl] -> [c, m]
        pc = points[:, :, nsl].transpose(1, 0, 2).reshape(C, M)
        in_maps.append(
            {
                "NP": np.ascontiguousarray(npc.astype(BF)),
                "NX": np.ascontiguousarray(nxc.astype(BF)),
                "XYZ": np.ascontiguousarray(xc.astype(BF)),
                "P": np.ascontiguousarray(pc.astype(BF)),
                "WK": Wk,
                "WKT": WkT,
                "WP": Wp,
                "HS": hs,
                "SELN": seln,
                "SELA": sela,
            }
        )
    return in_maps


def assemble_output(results):
    """Per-core OUT [128, 1024] bf16 (p, s, mi) -> full [B, N, S] f32."""
    out = np.empty((B, N, S), dtype=np.float32)
    for i in range(NCORES):
        oc = np.asarray(results[i]["OUT"]).astype(np.float32)
        oc = oc.reshape(128, S, 64).transpose(0, 2, 1).reshape(M, S)
        out[:, i * NL:(i + 1) * NL, :] = oc.reshape(B, NL, S)
    return out


def run_cores(in_maps, trace=False, trace_kwargs=None):
    nc = build_nc()
    return run_bass_kernel_spmd(
        nc,
        in_maps,
        core_ids=list(range(NCORES)),
        trace=trace,
        **(trace_kwargs or {}),
    )


def kernel(xyz, neighbor_xyz, points, neighbor_points, Wk, Wpos, bpos):
    in_maps = make_in_maps(
        xyz, neighbor_xyz, points, neighbor_points, Wk, Wpos, bpos
    )
    res = run_cores(in_maps, trace=False)
    return assemble_output(res.results)
